# revision 15
# baseline (speedup 1.0000x reference)
"""AttnDecoderRNN single-step decoder on 8 TRN2 NeuronCores.

Sharding:
  - Attention, embedding gather, LSTM: data-parallel over batch (8 rows
    per core); LSTM weights replicated (streamed bf16).
  - Output projection + log_softmax: vocab-sharded (12500 rows/core);
    AllGather of h_new, AllGather of partial sum(exp(logits)).
Numerics: attention score path fp32 (the gaussian-scaled softmax
amplifies score errors exponentially); gate and logit matmuls bf16 with
fp32 PSUM accumulation.
Host side only reshapes/slices/transposes inputs and concatenates outputs.
"""

import numpy as np
import ml_dtypes

import concourse.bass as bass
import concourse.bacc as bacc
import concourse.mybir as mybir
import concourse.tile as tile
from concourse.bass_utils import run_bass_kernel_spmd

H = 512
V = 100000
L = 15
B = 64
D = 5.0
NCORES = 8
BSH = B // NCORES       # 8 batch rows per core
VSH = V // NCORES       # 12500 vocab rows per core
NT = 25                 # vocab tiles per core
TN = VSH // NT          # 500 vocab cols per tile
LB = L * BSH            # 120

F32 = mybir.dt.float32
BF16 = mybir.dt.bfloat16
I32 = mybir.dt.int32
AX = mybir.AxisListType
AF = mybir.ActivationFunctionType
OP = mybir.AluOpType

GAUSS_SCALE = 1.0 / (D / 2.0) ** 2  # 0.16


def build_graph():
    nc = bacc.Bacc(
        "TRN2", target_bir_lowering=False, debug=False, num_devices=NCORES
    )
    rg = [list(range(NCORES))]

    p_ids = nc.declare_dram_parameter("ids", [BSH, 1], I32, isOutput=False)
    p_emb = nc.declare_dram_parameter("emb", [V, H], F32, isOutput=False)
    p_xatt = nc.declare_dram_parameter("xatt", [3 * H, LB], F32, isOutput=False)
    p_uawa = nc.declare_dram_parameter("uawa", [3 * H, H], F32, isOutput=False)
    p_ba = nc.declare_dram_parameter("ba", [1, H], F32, isOutput=False)
    p_wp = nc.declare_dram_parameter("wp", [H, H], F32, isOutput=False)
    p_vp = nc.declare_dram_parameter("vp", [1, H], F32, isOutput=False)
    p_hT = nc.declare_dram_parameter("hT", [H, BSH], F32, isOutput=False)
    p_hsh = nc.declare_dram_parameter("hsh", [BSH, H], F32, isOutput=False)
    p_ectx = nc.declare_dram_parameter("ectx", [LB, 2 * H], F32, isOutput=False)
    p_wcat = nc.declare_dram_parameter("wcat", [4 * H, 4 * H], BF16, isOutput=False)
    p_bih = nc.declare_dram_parameter("bih", [1, 4 * H], F32, isOutput=False)
    p_bhh = nc.declare_dram_parameter("bhh", [1, 4 * H], F32, isOutput=False)
    p_csh = nc.declare_dram_parameter("csh", [BSH, H], F32, isOutput=False)
    p_wT = nc.declare_dram_parameter("wT", [NT * 128, 4 * TN], BF16, isOutput=False)
    p_outb = nc.declare_dram_parameter("outb", [1, VSH], BF16, isOutput=False)
    p_pos = nc.declare_dram_parameter("pos", [1, L], F32, isOutput=False)
    p_ident = nc.declare_dram_parameter("ident", [128, 128], F32, isOutput=False)
    p_mask = nc.declare_dram_parameter("mask", [LB, BSH], F32, isOutput=False)

    o_logp = nc.declare_dram_parameter("o_logp", [B, VSH], F32, isOutput=True)
    o_h = nc.declare_dram_parameter("o_h", [BSH, H], F32, isOutput=True)
    o_c = nc.declare_dram_parameter("o_c", [BSH, H], F32, isOutput=True)
    o_attn = nc.declare_dram_parameter("o_attn", [BSH, L], F32, isOutput=True)

    with tile.TileContext(nc) as tc:
        with (
            tc.tile_pool(name="const", bufs=1) as cpool,
            tc.tile_pool(name="small", bufs=1) as spool,
            tc.tile_pool(name="wtiles", bufs=10) as wpool,
            tc.tile_pool(name="wk", bufs=3) as wkpool,
            tc.tile_pool(name="otiles", bufs=3) as opool,
            tc.tile_pool(name="obias", bufs=2) as obpool,
            tc.tile_pool(name="psatt", bufs=1, space="PSUM") as psA,
            tc.tile_pool(name="pstr", bufs=2, space="PSUM") as psT,
            tc.tile_pool(name="psg", bufs=2, space="PSUM") as psG,
            tc.tile_pool(name="pslog", bufs=2, space="PSUM") as psL,
            tc.tile_pool(name="dram", bufs=1, space="DRAM") as dpool,
        ):
            # ---------- constants / small loads ----------
            s_ident = cpool.tile([128, 128], F32)
            nc.sync.dma_start(s_ident[:], p_ident[:])
            s_ids = cpool.tile([BSH, 1], I32)
            nc.sync.dma_start(s_ids[:], p_ids[:])
            s_ba = cpool.tile([LB, H], F32)
            nc.sync.dma_start(s_ba[:], p_ba[:].to_broadcast([LB, H]))
            s_vp = cpool.tile([BSH, H], F32)
            nc.sync.dma_start(s_vp[:], p_vp[:].to_broadcast([BSH, H]))
            s_pos = cpool.tile([BSH, L], F32)
            nc.sync.dma_start(s_pos[:], p_pos[:].to_broadcast([BSH, L]))
            s_M = cpool.tile([LB, BSH], F32)
            nc.sync.dma_start(s_M[:], p_mask[:])

            # ---------- attention scores (fp32) ----------
            s_xatt = cpool.tile([128, 12 * LB], F32)
            nc.sync.dma_start(
                s_xatt[:].rearrange("p (k n) -> p k n", k=12),
                p_xatt[:].rearrange("(k p) n -> p k n", p=128),
            )
            ps_att = psA.tile([LB, H], F32, tag="att")
            for k in range(12):
                s_uawa = wkpool.tile([128, H], F32, tag="uawa")
                nc.sync.dma_start(s_uawa[:], p_uawa[k * 128 : (k + 1) * 128, :])
                nc.tensor.matmul(
                    ps_att[:],
                    s_xatt[:, k * LB : (k + 1) * LB],
                    s_uawa[:],
                    start=(k == 0),
                    stop=(k == 11),
                )
            s_tanh = spool.tile([LB, H], F32)
            nc.scalar.activation(s_tanh[:], ps_att[:], AF.Tanh)
            s_tscr = spool.tile([LB, H], F32)
            nc.vector.tensor_tensor(
                out=s_tscr[:], in0=s_tanh[:], in1=s_ba[:], op=OP.mult
            )
            s_scores = spool.tile([LB, 1], F32)
            nc.vector.tensor_reduce(
                out=s_scores[:], in_=s_tscr[:], axis=AX.X, op=OP.add
            )
            d_sc = dpool.tile([LB, 1], F32)
            nc.sync.dma_start(d_sc[:], s_scores[:])
            s_sc = spool.tile([BSH, L], F32)
            nc.sync.dma_start(s_sc[:], d_sc[:].rearrange("(b l) o -> b (l o)", b=BSH))

            # ---------- p_t and gaussian ----------
            s_hT = cpool.tile([128, 4 * BSH], F32)
            nc.sync.dma_start(
                s_hT[:].rearrange("p (k n) -> p k n", k=4),
                p_hT[:].rearrange("(k p) n -> p k n", p=128),
            )
            ps_wp = psT.tile([BSH, H], F32, tag="tr")
            for k in range(4):
                s_wpt = wkpool.tile([128, H], F32, tag="uawa")
                nc.sync.dma_start(s_wpt[:], p_wp[k * 128 : (k + 1) * 128, :])
                nc.tensor.matmul(
                    ps_wp[:],
                    s_hT[:, k * BSH : (k + 1) * BSH],
                    s_wpt[:],
                    start=(k == 0),
                    stop=(k == 3),
                )
            s_tanh2 = spool.tile([BSH, H], F32)
            nc.scalar.activation(s_tanh2[:], ps_wp[:], AF.Tanh)
            s_tscr2 = spool.tile([BSH, H], F32)
            nc.vector.tensor_tensor(
                out=s_tscr2[:], in0=s_tanh2[:], in1=s_vp[:], op=OP.mult
            )
            s_dot = spool.tile([BSH, 1], F32)
            nc.vector.tensor_reduce(
                out=s_dot[:], in_=s_tscr2[:], axis=AX.X, op=OP.add
            )
            s_pt = spool.tile([BSH, 1], F32)
            nc.scalar.activation(s_pt[:], s_dot[:], AF.Sigmoid)
            nc.vector.tensor_scalar_mul(s_pt[:], s_pt[:], float(L))
            s_diff = spool.tile([BSH, L], F32)
            nc.vector.tensor_scalar(
                out=s_diff[:], in0=s_pos[:], scalar1=s_pt[:], scalar2=None,
                op0=OP.subtract,
            )
            s_sq = spool.tile([BSH, L], F32)
            nc.scalar.activation(s_sq[:], s_diff[:], AF.Square)
            s_gw = spool.tile([BSH, L], F32)
            nc.scalar.activation(s_gw[:], s_sq[:], AF.Exp, scale=GAUSS_SCALE)
            s_w = spool.tile([BSH, L], F32)
            nc.vector.tensor_tensor(out=s_w[:], in0=s_sc[:], in1=s_gw[:], op=OP.mult)
            s_negm = spool.tile([BSH, 1], F32)
            nc.vector.tensor_reduce(
                out=s_negm[:], in_=s_w[:], axis=AX.X, op=OP.max, negate=True
            )
            s_ew = spool.tile([BSH, L], F32)
            s_wsum = spool.tile([BSH, 1], F32)
            nc.scalar.activation(
                s_ew[:], s_w[:], AF.Exp, bias=s_negm[:], accum_out=s_wsum[:]
            )
            s_rs = spool.tile([BSH, 1], F32)
            nc.vector.reciprocal(s_rs[:], s_wsum[:])
            s_attnw = spool.tile([BSH, L], F32)
            nc.vector.tensor_scalar(
                out=s_attnw[:], in0=s_ew[:], scalar1=s_rs[:], scalar2=None,
                op0=OP.mult,
            )
            nc.sync.dma_start(o_attn[:], s_attnw[:])

            # ---------- attn_out = attn_w @ enc_ctx (block-diag trick) ----------
            d_aw = dpool.tile([BSH, L], F32)
            nc.sync.dma_start(d_aw[:], s_attnw[:])
            s_awf = spool.tile([LB, 1], F32)
            nc.sync.dma_start(
                s_awf[:], d_aw[:].rearrange("b (l o) -> (b l) o", o=1)
            )
            s_A = spool.tile([LB, BSH], F32)
            nc.vector.tensor_scalar(
                out=s_A[:], in0=s_M[:], scalar1=s_awf[:], scalar2=None, op0=OP.mult
            )
            s_ectx = cpool.tile([LB, 2 * H], F32)
            nc.sync.dma_start(s_ectx[:], p_ectx[:])

            # x_loc = [embed | attn_out | h]  [8, 2048]
            s_xloc = spool.tile([BSH, 4 * H], F32)
            nc.gpsimd.indirect_dma_start(
                out=s_xloc[:, 0:H],
                out_offset=None,
                in_=p_emb[:],
                in_offset=bass.IndirectOffsetOnAxis(ap=s_ids[:, :1], axis=0),
            )
            for ns in range(2):
                ps_ao = psT.tile([BSH, H], F32, tag="tr")
                nc.tensor.matmul(
                    ps_ao[:],
                    s_A[:],
                    s_ectx[:, ns * H : (ns + 1) * H],
                    start=True,
                    stop=True,
                )
                nc.vector.tensor_copy(
                    s_xloc[:, (1 + ns) * H : (2 + ns) * H], ps_ao[:]
                )
            nc.sync.dma_start(s_xloc[:, 3 * H : 4 * H], p_hsh[:])

            # transpose x_loc -> bf16 lhsT chunks [128, 16*8]
            s_xgT = spool.tile([128, 16 * BSH], BF16)
            for j in range(16):
                ps_t = psT.tile([128, BSH], F32, tag="tr")
                nc.tensor.transpose(
                    ps_t[:], s_xloc[:, j * 128 : (j + 1) * 128],
                    s_ident[:BSH, :BSH],
                )
                nc.vector.tensor_copy(s_xgT[:, j * BSH : (j + 1) * BSH], ps_t[:])

            # ---------- gates = x_loc @ Wcat (bf16, batch-sharded) ----------
            s_bi = spool.tile([1, 4 * H], F32)
            nc.sync.dma_start(s_bi[:], p_bih[:])
            s_bh = spool.tile([1, 4 * H], F32)
            nc.sync.dma_start(s_bh[:], p_bhh[:])
            s_b1 = spool.tile([1, 4 * H], F32)
            nc.vector.tensor_tensor(
                out=s_b1[:], in0=s_bi[:], in1=s_bh[:], op=OP.add
            )
            d_bias = dpool.tile([1, 4 * H], F32)
            nc.sync.dma_start(d_bias[:], s_b1[:])
            s_csh = spool.tile([BSH, H], F32)
            nc.sync.dma_start(s_csh[:], p_csh[:])

            s_gates = spool.tile([BSH, 4 * H], F32)
            for nt in range(4):
                gsl = slice(nt * H, (nt + 1) * H)
                ps_g = psG.tile([BSH, H], F32, tag="g")
                for k in range(16):
                    s_wct = wkpool.tile([128, H], BF16, tag="wcat")
                    nc.sync.dma_start(
                        s_wct[:], p_wcat[k * 128 : (k + 1) * 128, gsl]
                    )
                    nc.tensor.matmul(
                        ps_g[:],
                        s_xgT[:, k * BSH : (k + 1) * BSH],
                        s_wct[:],
                        start=(k == 0),
                        stop=(k == 15),
                    )
                s_bgt = obpool.tile([BSH, H], F32, tag="bg")
                nc.sync.dma_start(
                    s_bgt[:], d_bias[:, gsl].to_broadcast([BSH, H])
                )
                nc.vector.tensor_tensor(
                    out=s_gates[:, gsl], in0=ps_g[:], in1=s_bgt[:], op=OP.add
                )

            # ---------- LSTM cell ([8, 512]) ----------
            s_si = spool.tile([BSH, H], F32)
            s_sf = spool.tile([BSH, H], F32)
            s_tg = spool.tile([BSH, H], F32)
            s_so = spool.tile([BSH, H], F32)
            nc.scalar.activation(s_si[:], s_gates[:, 0:H], AF.Sigmoid)
            nc.scalar.activation(s_sf[:], s_gates[:, H : 2 * H], AF.Sigmoid)
            nc.scalar.activation(s_tg[:], s_gates[:, 2 * H : 3 * H], AF.Tanh)
            nc.scalar.activation(s_so[:], s_gates[:, 3 * H : 4 * H], AF.Sigmoid)
            s_cnew = spool.tile([BSH, H], F32)
            nc.vector.tensor_tensor(out=s_sf[:], in0=s_sf[:], in1=s_csh[:], op=OP.mult)
            nc.vector.tensor_tensor(out=s_si[:], in0=s_si[:], in1=s_tg[:], op=OP.mult)
            nc.vector.tensor_tensor(out=s_cnew[:], in0=s_sf[:], in1=s_si[:], op=OP.add)
            s_tc = spool.tile([BSH, H], F32)
            nc.scalar.activation(s_tc[:], s_cnew[:], AF.Tanh)
            s_hnew = spool.tile([BSH, H], F32)
            nc.vector.tensor_tensor(out=s_hnew[:], in0=s_so[:], in1=s_tc[:], op=OP.mult)
            nc.sync.dma_start(o_c[:], s_cnew[:])
            nc.sync.dma_start(o_h[:], s_hnew[:])

            # ---------- AllGather h_new -> [64, 512], transpose to bf16 lhsT ----------
            d_agh_in = dpool.tile([BSH, H], F32)
            d_agh_out = dpool.tile([B, H], F32)
            nc.sync.dma_start(d_agh_in[:], s_hnew[:])
            nc.gpsimd.collective_compute(
                "AllGather",
                OP.bypass,
                replica_groups=rg,
                ins=[d_agh_in.opt()],
                outs=[d_agh_out.opt()],
            )
            s_hf = spool.tile([B, H], F32)
            nc.sync.dma_start(s_hf[:], d_agh_out[:])
            s_hTf = spool.tile([128, 4 * B], BF16)
            for k in range(4):
                ps_ht = psT.tile([128, B], F32, tag="tr")
                nc.tensor.transpose(
                    ps_ht[:], s_hf[:, k * 128 : (k + 1) * 128], s_ident[:B, :B]
                )
                nc.vector.tensor_copy(s_hTf[:, k * B : (k + 1) * B], ps_ht[:])

            # ---------- logits: stream wT tiles (bf16), exp-accumulate ----------
            s_ones = cpool.tile([1, B], BF16)
            nc.vector.memset(s_ones[:], 1.0)
            s_logits = spool.tile([B, VSH], BF16)
            s_part = spool.tile([B, NT], F32)
            for t in range(NT):
                sl = slice(t * TN, (t + 1) * TN)
                s_wt = wpool.tile([128, 4 * TN], BF16, tag="wt")
                nc.sync.dma_start(s_wt[:], p_wT[t * 128 : (t + 1) * 128, :])
                s_ob = obpool.tile([1, TN], BF16, tag="ob")
                nc.sync.dma_start(s_ob[:], p_outb[:, sl])
                ps_l = psL.tile([B, TN], F32, tag="log")
                for k in range(4):
                    nc.tensor.matmul(
                        ps_l[:],
                        s_hTf[:, k * B : (k + 1) * B],
                        s_wt[:, k * TN : (k + 1) * TN],
                        start=(k == 0),
                        stop=False,
                    )
                nc.tensor.matmul(ps_l[:], s_ones[:], s_ob[:], start=False, stop=True)
                nc.vector.tensor_copy(s_logits[:, sl], ps_l[:])
                s_e = opool.tile([B, TN], F32, tag="e")
                nc.scalar.activation(
                    s_e[:], ps_l[:], AF.Exp, accum_out=s_part[:, t : t + 1]
                )
            s_sum = spool.tile([B, 1], F32)
            nc.vector.tensor_reduce(out=s_sum[:], in_=s_part[:], axis=AX.X, op=OP.add)

            # ---------- AllGather partial sums, reduce locally ----------
            d_ar_in = dpool.tile([B, 1], F32)
            d_ar_out = dpool.tile([NCORES * B, 1], F32)
            nc.sync.dma_start(d_ar_in[:], s_sum[:])
            nc.gpsimd.collective_compute(
                "AllGather",
                OP.bypass,
                replica_groups=rg,
                ins=[d_ar_in.opt()],
                outs=[d_ar_out.opt()],
            )
            s_parts = spool.tile([B, NCORES], F32)
            nc.sync.dma_start(
                s_parts[:], d_ar_out[:].rearrange("(r b) o -> b (r o)", r=NCORES)
            )
            s_tot = spool.tile([B, 1], F32)
            nc.vector.tensor_reduce(out=s_tot[:], in_=s_parts[:], axis=AX.X, op=OP.add)
            s_lse = spool.tile([B, 1], F32)
            nc.scalar.activation(s_lse[:], s_tot[:], AF.Ln)

            # ---------- out = logits - lse ----------
            for t in range(NT):
                sl = slice(t * TN, (t + 1) * TN)
                s_ot = opool.tile([B, TN], F32, tag="ot")
                nc.vector.tensor_scalar(
                    out=s_ot[:], in0=s_logits[:, sl], scalar1=s_lse[:],
                    scalar2=None, op0=OP.subtract,
                )
                nc.sync.dma_start(o_logp[:, sl], s_ot[:])

    nc.compile()
    return nc


_NC_CACHE = None
LAST_RESULT = None


def _get_graph():
    global _NC_CACHE
    if _NC_CACHE is None:
        _NC_CACHE = build_graph()
    return _NC_CACHE


def kernel(input_ids, hidden, c, encoder_outputs, embedding, Wa, Ua, ba,
           W_p, V_p, W_ih, W_hh, b_ih, b_hh, out_W, out_b):
    f32 = np.float32
    bf16 = ml_dtypes.bfloat16
    ids = np.asarray(input_ids, np.int32)
    hidden = np.asarray(hidden, f32)
    c = np.asarray(c, f32)
    enc = np.asarray(encoder_outputs, f32)
    emb = np.ascontiguousarray(np.asarray(embedding, f32))
    Wa, Ua, ba = (np.asarray(a, f32) for a in (Wa, Ua, ba))
    W_p, V_p = np.asarray(W_p, f32), np.asarray(V_p, f32)
    W_ih, W_hh = np.asarray(W_ih, f32), np.asarray(W_hh, f32)
    b_ih, b_hh = np.asarray(b_ih, f32), np.asarray(b_hh, f32)
    out_W, out_b = np.asarray(out_W, f32), np.asarray(out_b, f32)

    h0 = hidden[0]
    c0 = c[0]
    enc_flat = enc.reshape(B * L, 2 * H)
    uawa = np.ascontiguousarray(np.concatenate([Ua, Wa], axis=0))
    wcat = np.ascontiguousarray(
        np.concatenate([W_ih, W_hh], axis=1).T
    ).astype(bf16)
    ident = np.eye(128, dtype=f32)
    pos = np.arange(L, dtype=f32)[None, :]
    blkmask = np.zeros((LB, BSH), f32)
    for b in range(BSH):
        blkmask[b * L : (b + 1) * L, b] = 1.0
    bih_r = np.ascontiguousarray(b_ih[None, :])
    bhh_r = np.ascontiguousarray(b_hh[None, :])
    ba_r = np.ascontiguousarray(ba[None, :])
    vp_r = np.ascontiguousarray(V_p[None, :])

    in_maps = []
    for i in range(NCORES):
        bs = slice(i * BSH, (i + 1) * BSH)
        vs = slice(i * VSH, (i + 1) * VSH)
        wt_il = np.ascontiguousarray(
            out_W[vs].T.reshape(4, 128, NT, TN).transpose(2, 1, 0, 3)
            .reshape(NT * 128, 4 * TN).astype(bf16)
        )
        enc_att = enc[:, bs, :]
        Xa = np.empty((LB, 3 * H), f32)
        for b in range(BSH):
            for l in range(L):
                Xa[b * L + l, : 2 * H] = enc_att[l, b]
                Xa[b * L + l, 2 * H :] = h0[i * BSH + b]
        in_maps.append({
            "ids": np.ascontiguousarray(ids[0, bs].reshape(BSH, 1)),
            "emb": emb,
            "xatt": np.ascontiguousarray(Xa.T),
            "uawa": uawa,
            "ba": ba_r,
            "wp": W_p,
            "vp": vp_r,
            "hT": np.ascontiguousarray(h0[bs].T),
            "hsh": np.ascontiguousarray(h0[bs]),
            "ectx": np.ascontiguousarray(enc_flat[i * LB : (i + 1) * LB]),
            "wcat": wcat,
            "bih": bih_r,
            "bhh": bhh_r,
            "csh": np.ascontiguousarray(c0[bs]),
            "wT": wt_il,
            "outb": np.ascontiguousarray(out_b[vs][None, :]).astype(bf16),
            "pos": pos,
            "ident": ident,
            "mask": blkmask,
        })

    nc = _get_graph()
    res = run_bass_kernel_spmd(nc, in_maps, core_ids=list(range(NCORES)))
    global LAST_RESULT
    LAST_RESULT = res
    outs = res.results

    logp = np.concatenate([outs[i]["o_logp"] for i in range(NCORES)], axis=1)
    h_new = np.concatenate([outs[i]["o_h"] for i in range(NCORES)], axis=0)
    c_new = np.concatenate([outs[i]["o_c"] for i in range(NCORES)], axis=0)
    attn = np.concatenate([outs[i]["o_attn"] for i in range(NCORES)], axis=0)
    return (
        logp.astype(f32),
        h_new[None].astype(f32),
        c_new[None].astype(f32),
        attn[:, None, :].astype(f32),
    )


# revision 20
# speedup vs baseline: 1.2073x; 1.2073x over previous
"""AttnDecoderRNN single-step decoder on 8 TRN2 NeuronCores.

Sharding:
  - Attention, embedding gather, LSTM: data-parallel over batch (8 rows
    per core); LSTM weights replicated (streamed bf16).
  - Output projection + log_softmax: vocab-sharded (12500 rows/core);
    AllGather of h_new, AllGather of partial sum(exp(logits)).
Numerics: attention score path fp32 (the gaussian-scaled softmax
amplifies score errors exponentially); gate and logit matmuls bf16 with
fp32 PSUM accumulation.
Host side only reshapes/slices/transposes inputs and concatenates outputs.
"""

import numpy as np
import ml_dtypes

import concourse.bass as bass
import concourse.bacc as bacc
import concourse.mybir as mybir
import concourse.tile as tile
from concourse.bass_utils import run_bass_kernel_spmd

H = 512
V = 100000
L = 15
B = 64
D = 5.0
NCORES = 8
BSH = B // NCORES       # 8 batch rows per core
VSH = V // NCORES       # 12500 vocab rows per core
NT = 25                 # vocab tiles per core
TN = VSH // NT          # 500 vocab cols per tile
LB = L * BSH            # 120

F32 = mybir.dt.float32
BF16 = mybir.dt.bfloat16
F8 = mybir.dt.float8e4
I32 = mybir.dt.int32
AX = mybir.AxisListType
AF = mybir.ActivationFunctionType
OP = mybir.AluOpType

GAUSS_SCALE = 1.0 / (D / 2.0) ** 2  # 0.16
WSCALE = 16.0


def build_graph():
    nc = bacc.Bacc(
        "TRN2", target_bir_lowering=False, debug=False, num_devices=NCORES
    )
    rg = [list(range(NCORES))]

    p_ids = nc.declare_dram_parameter("ids", [BSH, 1], I32, isOutput=False)
    p_emb = nc.declare_dram_parameter("emb", [V, H], F32, isOutput=False)
    p_xatt = nc.declare_dram_parameter("xatt", [3 * H, LB], F32, isOutput=False)
    p_uawa = nc.declare_dram_parameter("uawa", [3 * 128, 4 * H], F32, isOutput=False)
    p_ba = nc.declare_dram_parameter("ba", [1, H], F32, isOutput=False)
    p_wp = nc.declare_dram_parameter("wp", [128, 4 * H], F32, isOutput=False)
    p_vp = nc.declare_dram_parameter("vp", [1, H], F32, isOutput=False)
    p_hT = nc.declare_dram_parameter("hT", [H, BSH], F32, isOutput=False)
    p_hsh = nc.declare_dram_parameter("hsh", [BSH, H], F32, isOutput=False)
    p_ectx = nc.declare_dram_parameter("ectx", [LB, 2 * H], F32, isOutput=False)
    p_wcat = nc.declare_dram_parameter("wcat", [4 * H, 4 * H], BF16, isOutput=False)
    p_bih = nc.declare_dram_parameter("bih", [1, 4 * H], F32, isOutput=False)
    p_bhh = nc.declare_dram_parameter("bhh", [1, 4 * H], F32, isOutput=False)
    p_csh = nc.declare_dram_parameter("csh", [BSH, H], F32, isOutput=False)
    p_wT = nc.declare_dram_parameter("wT", [NT * 128, 4 * TN], F8, isOutput=False)
    p_outb = nc.declare_dram_parameter("outb", [1, VSH], BF16, isOutput=False)
    p_pos = nc.declare_dram_parameter("pos", [1, L], F32, isOutput=False)
    p_ident = nc.declare_dram_parameter("ident", [128, 128], F32, isOutput=False)
    p_mask = nc.declare_dram_parameter("mask", [LB, BSH], F32, isOutput=False)

    o_logp = nc.declare_dram_parameter("o_logp", [B, VSH], F32, isOutput=True)
    o_h = nc.declare_dram_parameter("o_h", [BSH, H], F32, isOutput=True)
    o_c = nc.declare_dram_parameter("o_c", [BSH, H], F32, isOutput=True)
    o_attn = nc.declare_dram_parameter("o_attn", [BSH, L], F32, isOutput=True)

    with tile.TileContext(nc) as tc:
        with (
            tc.tile_pool(name="const", bufs=1) as cpool,
            tc.tile_pool(name="small", bufs=1) as spool,
            tc.tile_pool(name="wtiles", bufs=8) as wpool,
            tc.tile_pool(name="wk", bufs=3) as wkpool,
            tc.tile_pool(name="otiles", bufs=2) as opool,
            tc.tile_pool(name="obias", bufs=2) as obpool,
            tc.tile_pool(name="psatt", bufs=1, space="PSUM") as psA,
            tc.tile_pool(name="pstr", bufs=2, space="PSUM") as psT,
            tc.tile_pool(name="psmm", bufs=4, space="PSUM") as psM,
            tc.tile_pool(name="dram", bufs=1, space="DRAM") as dpool,
        ):
            # ---------- constants / small loads ----------
            s_ident = cpool.tile([128, 128], F32)
            nc.sync.dma_start(s_ident[:], p_ident[:])
            s_ids = cpool.tile([BSH, 1], I32)
            nc.sync.dma_start(s_ids[:], p_ids[:])
            s_ba = cpool.tile([LB, H], F32)
            nc.sync.dma_start(s_ba[:], p_ba[:].to_broadcast([LB, H]))
            s_vp = cpool.tile([BSH, H], F32)
            nc.sync.dma_start(s_vp[:], p_vp[:].to_broadcast([BSH, H]))
            s_pos = cpool.tile([BSH, L], F32)
            nc.sync.dma_start(s_pos[:], p_pos[:].to_broadcast([BSH, L]))
            s_M = cpool.tile([LB, BSH], F32)
            nc.sync.dma_start(s_M[:], p_mask[:])

            # ---------- attention scores (fp32) ----------
            s_xatt = cpool.tile([128, 12 * LB], F32)
            nc.sync.dma_start(
                s_xatt[:].rearrange("p (k n) -> p k n", k=12),
                p_xatt[:].rearrange("(k p) n -> p k n", p=128),
            )
            ps_att = psA.tile([LB, H], F32, tag="att")
            for c in range(3):
                s_uawa = wkpool.tile([128, 4 * H], F32, tag="uawa")
                nc.sync.dma_start(
                    s_uawa[:], p_uawa[c * 128 : (c + 1) * 128, :]
                )
                for j in range(4):
                    k = 4 * c + j
                    nc.tensor.matmul(
                        ps_att[:],
                        s_xatt[:, k * LB : (k + 1) * LB],
                        s_uawa[:, j * H : (j + 1) * H],
                        start=(k == 0),
                        stop=(k == 11),
                    )
            s_tanh = spool.tile([LB, H], F32)
            nc.scalar.activation(s_tanh[:], ps_att[:], AF.Tanh)
            s_tscr = spool.tile([LB, H], F32)
            nc.vector.tensor_tensor(
                out=s_tscr[:], in0=s_tanh[:], in1=s_ba[:], op=OP.mult
            )
            s_scores = spool.tile([LB, 1], F32)
            nc.vector.tensor_reduce(
                out=s_scores[:], in_=s_tscr[:], axis=AX.X, op=OP.add
            )
            d_sc = dpool.tile([LB, 1], F32)
            nc.sync.dma_start(d_sc[:], s_scores[:])
            s_sc = spool.tile([BSH, L], F32)
            nc.sync.dma_start(s_sc[:], d_sc[:].rearrange("(b l) o -> b (l o)", b=BSH))

            # ---------- p_t and gaussian ----------
            s_hT = cpool.tile([128, 4 * BSH], F32)
            nc.sync.dma_start(
                s_hT[:].rearrange("p (k n) -> p k n", k=4),
                p_hT[:].rearrange("(k p) n -> p k n", p=128),
            )
            ps_wp = psT.tile([BSH, H], F32, tag="tr")
            s_wpt = wkpool.tile([128, 4 * H], F32, tag="uawa")
            nc.sync.dma_start(s_wpt[:], p_wp[:])
            for k in range(4):
                nc.tensor.matmul(
                    ps_wp[:],
                    s_hT[:, k * BSH : (k + 1) * BSH],
                    s_wpt[:, k * H : (k + 1) * H],
                    start=(k == 0),
                    stop=(k == 3),
                )
            s_tanh2 = spool.tile([BSH, H], F32)
            nc.scalar.activation(s_tanh2[:], ps_wp[:], AF.Tanh)
            s_tscr2 = spool.tile([BSH, H], F32)
            nc.vector.tensor_tensor(
                out=s_tscr2[:], in0=s_tanh2[:], in1=s_vp[:], op=OP.mult
            )
            s_dot = spool.tile([BSH, 1], F32)
            nc.vector.tensor_reduce(
                out=s_dot[:], in_=s_tscr2[:], axis=AX.X, op=OP.add
            )
            s_pt = spool.tile([BSH, 1], F32)
            nc.scalar.activation(s_pt[:], s_dot[:], AF.Sigmoid)
            nc.vector.tensor_scalar_mul(s_pt[:], s_pt[:], float(L))
            s_diff = spool.tile([BSH, L], F32)
            nc.vector.tensor_scalar(
                out=s_diff[:], in0=s_pos[:], scalar1=s_pt[:], scalar2=None,
                op0=OP.subtract,
            )
            s_sq = spool.tile([BSH, L], F32)
            nc.scalar.activation(s_sq[:], s_diff[:], AF.Square)
            s_gw = spool.tile([BSH, L], F32)
            nc.scalar.activation(s_gw[:], s_sq[:], AF.Exp, scale=GAUSS_SCALE)
            s_w = spool.tile([BSH, L], F32)
            nc.vector.tensor_tensor(out=s_w[:], in0=s_sc[:], in1=s_gw[:], op=OP.mult)
            s_negm = spool.tile([BSH, 1], F32)
            nc.vector.tensor_reduce(
                out=s_negm[:], in_=s_w[:], axis=AX.X, op=OP.max, negate=True
            )
            s_ew = spool.tile([BSH, L], F32)
            s_wsum = spool.tile([BSH, 1], F32)
            nc.scalar.activation(
                s_ew[:], s_w[:], AF.Exp, bias=s_negm[:], accum_out=s_wsum[:]
            )
            s_rs = spool.tile([BSH, 1], F32)
            nc.vector.reciprocal(s_rs[:], s_wsum[:])
            s_attnw = spool.tile([BSH, L], F32)
            nc.vector.tensor_scalar(
                out=s_attnw[:], in0=s_ew[:], scalar1=s_rs[:], scalar2=None,
                op0=OP.mult,
            )
            nc.sync.dma_start(o_attn[:], s_attnw[:])

            # ---------- attn_out = attn_w @ enc_ctx (block-diag trick) ----------
            d_aw = dpool.tile([BSH, L], F32)
            nc.sync.dma_start(d_aw[:], s_attnw[:])
            s_awf = spool.tile([LB, 1], F32)
            nc.sync.dma_start(
                s_awf[:], d_aw[:].rearrange("b (l o) -> (b l) o", o=1)
            )
            s_A = spool.tile([LB, BSH], F32)
            nc.vector.tensor_scalar(
                out=s_A[:], in0=s_M[:], scalar1=s_awf[:], scalar2=None, op0=OP.mult
            )
            s_ectx = cpool.tile([LB, 2 * H], F32)
            nc.sync.dma_start(s_ectx[:], p_ectx[:])

            # x_loc = [embed | attn_out | h]  [8, 2048]
            s_xloc = spool.tile([BSH, 4 * H], F32)
            nc.gpsimd.indirect_dma_start(
                out=s_xloc[:, 0:H],
                out_offset=None,
                in_=p_emb[:],
                in_offset=bass.IndirectOffsetOnAxis(ap=s_ids[:, :1], axis=0),
            )
            for ns in range(2):
                ps_ao = psT.tile([BSH, H], F32, tag="tr")
                nc.tensor.matmul(
                    ps_ao[:],
                    s_A[:],
                    s_ectx[:, ns * H : (ns + 1) * H],
                    start=True,
                    stop=True,
                )
                nc.vector.tensor_copy(
                    s_xloc[:, (1 + ns) * H : (2 + ns) * H], ps_ao[:]
                )
            nc.sync.dma_start(s_xloc[:, 3 * H : 4 * H], p_hsh[:])

            # transpose x_loc -> bf16 lhsT chunks [128, 16*8]
            s_xgT = spool.tile([128, 16 * BSH], BF16)
            for j in range(16):
                ps_t = psT.tile([128, BSH], F32, tag="tr")
                nc.tensor.transpose(
                    ps_t[:], s_xloc[:, j * 128 : (j + 1) * 128],
                    s_ident[:BSH, :BSH],
                )
                nc.vector.tensor_copy(s_xgT[:, j * BSH : (j + 1) * BSH], ps_t[:])

            # ---------- gates = x_loc @ Wcat (bf16, batch-sharded) ----------
            s_csh = spool.tile([BSH, H], F32)
            nc.sync.dma_start(s_csh[:], p_csh[:])

            s_gates = spool.tile([BSH, 4 * H], F32)
            ps_gs = [
                psM.tile([B, H], F32, tag="mm", name=f"ps_g{nt}")
                for nt in range(4)
            ]
            for k in range(16):
                s_wct = wkpool.tile([128, 4 * H], BF16, tag="wcat")
                nc.sync.dma_start(s_wct[:], p_wcat[k * 128 : (k + 1) * 128, :])
                for nt in range(4):
                    nc.tensor.matmul(
                        ps_gs[nt][:BSH, :],
                        s_xgT[:, k * BSH : (k + 1) * BSH],
                        s_wct[:, nt * H : (nt + 1) * H],
                        start=(k == 0),
                        stop=(k == 15),
                    )
            for nt in range(4):
                gsl = slice(nt * H, (nt + 1) * H)
                s_bgt = obpool.tile([BSH, H], F32, tag="bg")
                nc.sync.dma_start(s_bgt[:], p_bih[:, gsl].to_broadcast([BSH, H]))
                s_bgt2 = obpool.tile([BSH, H], F32, tag="bg")
                nc.sync.dma_start(s_bgt2[:], p_bhh[:, gsl].to_broadcast([BSH, H]))
                nc.vector.tensor_tensor(
                    out=s_bgt[:], in0=s_bgt[:], in1=s_bgt2[:], op=OP.add
                )
                nc.vector.tensor_tensor(
                    out=s_gates[:, gsl], in0=ps_gs[nt][:BSH, :], in1=s_bgt[:],
                    op=OP.add,
                )

            # ---------- LSTM cell ([8, 512]) ----------
            s_si = spool.tile([BSH, H], F32)
            s_sf = spool.tile([BSH, H], F32)
            s_tg = spool.tile([BSH, H], F32)
            s_so = spool.tile([BSH, H], F32)
            nc.scalar.activation(s_si[:], s_gates[:, 0:H], AF.Sigmoid)
            nc.scalar.activation(s_sf[:], s_gates[:, H : 2 * H], AF.Sigmoid)
            nc.scalar.activation(s_tg[:], s_gates[:, 2 * H : 3 * H], AF.Tanh)
            nc.scalar.activation(s_so[:], s_gates[:, 3 * H : 4 * H], AF.Sigmoid)
            s_cnew = spool.tile([BSH, H], F32)
            nc.vector.tensor_tensor(out=s_sf[:], in0=s_sf[:], in1=s_csh[:], op=OP.mult)
            nc.vector.tensor_tensor(out=s_si[:], in0=s_si[:], in1=s_tg[:], op=OP.mult)
            nc.vector.tensor_tensor(out=s_cnew[:], in0=s_sf[:], in1=s_si[:], op=OP.add)
            s_tc = spool.tile([BSH, H], F32)
            nc.scalar.activation(s_tc[:], s_cnew[:], AF.Tanh)
            s_hnew = spool.tile([BSH, H], F32)
            nc.vector.tensor_tensor(out=s_hnew[:], in0=s_so[:], in1=s_tc[:], op=OP.mult)
            nc.sync.dma_start(o_c[:], s_cnew[:])
            nc.sync.dma_start(o_h[:], s_hnew[:])

            # ---------- AllGather h_new -> [64, 512], transpose to bf16 lhsT ----------
            s_hnb = spool.tile([BSH, H], BF16)
            nc.vector.tensor_copy(s_hnb[:], s_hnew[:])
            d_agh_in = dpool.tile([BSH, H], BF16)
            d_agh_out = dpool.tile([B, H], BF16)
            nc.sync.dma_start(d_agh_in[:], s_hnb[:])
            nc.gpsimd.collective_compute(
                "AllGather",
                OP.bypass,
                replica_groups=rg,
                ins=[d_agh_in.opt()],
                outs=[d_agh_out.opt()],
            )
            s_hf = spool.tile([B, H], BF16)
            nc.sync.dma_start(s_hf[:], d_agh_out[:])
            s_identb = spool.tile([B, B], BF16)
            nc.vector.tensor_copy(s_identb[:], s_ident[:B, :B])
            s_hTf = spool.tile([128, 4 * B], F8)
            for k in range(4):
                ps_ht = psA.tile([128, B], BF16, tag="trb")
                nc.tensor.transpose(
                    ps_ht[:], s_hf[:, k * 128 : (k + 1) * 128], s_identb[:]
                )
                nc.vector.tensor_copy(s_hTf[:, k * B : (k + 1) * B], ps_ht[:])

            # ---------- logits: stream wT tiles (bf16), exp-accumulate ----------
            s_ones = cpool.tile([1, B], F8)
            nc.vector.memset(s_ones[:], 1.0)
            s_logits = spool.tile([B, VSH], BF16)
            s_part = spool.tile([B, NT], F32)
            for t in range(NT):
                sl = slice(t * TN, (t + 1) * TN)
                s_wt = wpool.tile([128, 4 * TN], F8, tag="wt")
                nc.sync.dma_start(s_wt[:], p_wT[t * 128 : (t + 1) * 128, :])
                s_ob = obpool.tile([1, TN], BF16, tag="ob")
                nc.sync.dma_start(s_ob[:], p_outb[:, sl])
                ps_l = psM.tile([B, TN], F32, tag="mm")
                for k in range(4):
                    nc.tensor.matmul(
                        ps_l[:],
                        s_hTf[:, k * B : (k + 1) * B],
                        s_wt[:, k * TN : (k + 1) * TN],
                        start=(k == 0),
                        stop=False,
                    )
                nc.tensor.matmul(ps_l[:], s_ones[:], s_ob[:], start=False, stop=True)
                nc.vector.tensor_scalar(
                    out=s_logits[:, sl], in0=ps_l[:], scalar1=1.0 / WSCALE,
                    scalar2=None, op0=OP.mult,
                )
                s_e = opool.tile([B, TN], F32, tag="e")
                nc.scalar.activation(
                    s_e[:], ps_l[:], AF.Exp, scale=1.0 / WSCALE,
                    accum_out=s_part[:, t : t + 1],
                )
            s_sum = spool.tile([B, 1], F32)
            nc.vector.tensor_reduce(out=s_sum[:], in_=s_part[:], axis=AX.X, op=OP.add)

            # ---------- AllGather partial sums, reduce locally ----------
            d_ar_in = dpool.tile([B, 1], F32)
            d_ar_out = dpool.tile([NCORES * B, 1], F32)
            nc.sync.dma_start(d_ar_in[:], s_sum[:])
            nc.gpsimd.collective_compute(
                "AllGather",
                OP.bypass,
                replica_groups=rg,
                ins=[d_ar_in.opt()],
                outs=[d_ar_out.opt()],
            )
            s_parts = spool.tile([B, NCORES], F32)
            nc.sync.dma_start(
                s_parts[:], d_ar_out[:].rearrange("(r b) o -> b (r o)", r=NCORES)
            )
            s_tot = spool.tile([B, 1], F32)
            nc.vector.tensor_reduce(out=s_tot[:], in_=s_parts[:], axis=AX.X, op=OP.add)
            s_lse = spool.tile([B, 1], F32)
            nc.scalar.activation(s_lse[:], s_tot[:], AF.Ln)

            # ---------- out = logits - lse ----------
            TW = VSH // 10
            for t in range(10):
                sl = slice(t * TW, (t + 1) * TW)
                s_ot = opool.tile([B, TW], F32, tag="ot")
                nc.vector.tensor_scalar(
                    out=s_ot[:], in0=s_logits[:, sl], scalar1=s_lse[:],
                    scalar2=None, op0=OP.subtract,
                )
                nc.sync.dma_start(o_logp[:, sl], s_ot[:])

    nc.compile()
    return nc


_NC_CACHE = None
LAST_RESULT = None


def _get_graph():
    global _NC_CACHE
    if _NC_CACHE is None:
        _NC_CACHE = build_graph()
    return _NC_CACHE


def kernel(input_ids, hidden, c, encoder_outputs, embedding, Wa, Ua, ba,
           W_p, V_p, W_ih, W_hh, b_ih, b_hh, out_W, out_b):
    f32 = np.float32
    bf16 = ml_dtypes.bfloat16
    ids = np.asarray(input_ids, np.int32)
    hidden = np.asarray(hidden, f32)
    c = np.asarray(c, f32)
    enc = np.asarray(encoder_outputs, f32)
    emb = np.ascontiguousarray(np.asarray(embedding, f32))
    Wa, Ua, ba = (np.asarray(a, f32) for a in (Wa, Ua, ba))
    W_p, V_p = np.asarray(W_p, f32), np.asarray(V_p, f32)
    W_ih, W_hh = np.asarray(W_ih, f32), np.asarray(W_hh, f32)
    b_ih, b_hh = np.asarray(b_ih, f32), np.asarray(b_hh, f32)
    out_W, out_b = np.asarray(out_W, f32), np.asarray(out_b, f32)

    h0 = hidden[0]
    c0 = c[0]
    enc_flat = enc.reshape(B * L, 2 * H)
    uawa_cat = np.concatenate([Ua, Wa], axis=0)
    uawa = np.ascontiguousarray(
        uawa_cat.reshape(3, 4, 128, H).transpose(0, 2, 1, 3).reshape(3 * 128, 4 * H)
    )
    wp_il = np.ascontiguousarray(
        W_p.reshape(4, 128, H).transpose(1, 0, 2).reshape(128, 4 * H)
    )
    wcat = np.ascontiguousarray(
        np.concatenate([W_ih, W_hh], axis=1).T
    ).astype(bf16)
    ident = np.eye(128, dtype=f32)
    pos = np.arange(L, dtype=f32)[None, :]
    blkmask = np.zeros((LB, BSH), f32)
    for b in range(BSH):
        blkmask[b * L : (b + 1) * L, b] = 1.0
    bih_r = np.ascontiguousarray(b_ih[None, :])
    bhh_r = np.ascontiguousarray(b_hh[None, :])
    ba_r = np.ascontiguousarray(ba[None, :])
    vp_r = np.ascontiguousarray(V_p[None, :])

    in_maps = []
    for i in range(NCORES):
        bs = slice(i * BSH, (i + 1) * BSH)
        vs = slice(i * VSH, (i + 1) * VSH)
        wt_il = np.ascontiguousarray(
            (out_W[vs].T * WSCALE).reshape(4, 128, NT, TN).transpose(2, 1, 0, 3)
            .reshape(NT * 128, 4 * TN).astype(ml_dtypes.float8_e4m3)
        )
        enc_att = enc[:, bs, :]
        Xa = np.empty((LB, 3 * H), f32)
        for b in range(BSH):
            for l in range(L):
                Xa[b * L + l, : 2 * H] = enc_att[l, b]
                Xa[b * L + l, 2 * H :] = h0[i * BSH + b]
        in_maps.append({
            "ids": np.ascontiguousarray(ids[0, bs].reshape(BSH, 1)),
            "emb": emb,
            "xatt": np.ascontiguousarray(Xa.T),
            "uawa": uawa,
            "ba": ba_r,
            "wp": wp_il,
            "vp": vp_r,
            "hT": np.ascontiguousarray(h0[bs].T),
            "hsh": np.ascontiguousarray(h0[bs]),
            "ectx": np.ascontiguousarray(enc_flat[i * LB : (i + 1) * LB]),
            "wcat": wcat,
            "bih": bih_r,
            "bhh": bhh_r,
            "csh": np.ascontiguousarray(c0[bs]),
            "wT": wt_il,
            "outb": np.ascontiguousarray(out_b[vs][None, :] * WSCALE).astype(bf16),
            "pos": pos,
            "ident": ident,
            "mask": blkmask,
        })

    nc = _get_graph()
    res = run_bass_kernel_spmd(nc, in_maps, core_ids=list(range(NCORES)))
    global LAST_RESULT
    LAST_RESULT = res
    outs = res.results

    logp = np.concatenate([outs[i]["o_logp"] for i in range(NCORES)], axis=1)
    h_new = np.concatenate([outs[i]["o_h"] for i in range(NCORES)], axis=0)
    c_new = np.concatenate([outs[i]["o_c"] for i in range(NCORES)], axis=0)
    attn = np.concatenate([outs[i]["o_attn"] for i in range(NCORES)], axis=0)
    return (
        logp.astype(f32),
        h_new[None].astype(f32),
        c_new[None].astype(f32),
        attn[:, None, :].astype(f32),
    )
